# revision 16
# baseline (speedup 1.0000x reference)
"""ChebNet (K=3, 2 ChebConv layers + node-halve maxpool + global max + FC +
log_softmax) on 8 Trainium2 NeuronCores — v2.

vs v1:
- bf16 gather tables: 256B rows both layers (halves gather DMA bytes)
- trailing -1 index padding: Q7 SWDGE trims pads, desc-gen ~ actual edges
- per-tile-slot static chunk counts (max over the 8 cores) instead of a
  global max -> less padding everywhere
- 4 SWDGE queues round-robin + 6 rotating gather buffers (parallel desc gen)
- bf16 staircase / matmuls / dense / collectives; f32 PSUM accumulation
- x1/x2/xp/l1 intermediates SBUF-resident (no DRAM round trips)
"""

import numpy as np
import ml_dtypes

import concourse.bass as bass
import concourse.bacc as bacc
import concourse.mybir as mybir
import concourse.tile as tile
import concourse.bass_utils as bass_utils
from concourse.library_config import mlp

F32 = mybir.dt.float32
BF16 = mybir.dt.bfloat16
I16 = mybir.dt.int16
I32 = mybir.dt.int32
AX = mybir.AxisListType
OP = mybir.AluOpType
AF = mybir.ActivationFunctionType
BF = ml_dtypes.bfloat16

P = 128
NQ = 4  # SWDGE queues


# --------------------------------------------------------------------------
# host-side graph preprocessing
# --------------------------------------------------------------------------

def _wrap_idx_block(vals):
    """Wrap flat int16 idx list (len % 128 == 0) into dma_gather SBUF layout:
    index i -> partition i % 16, column i // 16, replicated over 8 groups."""
    n = len(vals)
    blk = vals.reshape(n // 16, 16).T
    return np.tile(blk, (8, 1))


def _prep_layer(src, dst, N, NC, pair):
    """Sort edges by dst, bucket into 128-dst tiles, chunk by 128 edges.

    Chunk counts per tile-slot are the max over the NC cores (one SPMD
    program), but each core pads its own tail with idx -1 (Q7-trimmed).

    Returns (CHt, per_core): CHt[t] = chunks for tile-slot t;
    per_core[c] = {idx: [128, C*8] i16, dl: [128, C*DLW] bf16} with
    C = sum(CHt), DLW = 2 if pair else 1.
    """
    SH = N // NC
    T = SH // P
    DLW = 2 if pair else 1
    order = np.argsort(dst, kind="stable")
    ds = dst[order]
    ss = src[order].astype(np.int64)
    bounds = np.searchsorted(ds, np.arange(0, N + 1, P))
    tiles = []
    for g in range(N // P):
        tiles.append((ss[bounds[g]:bounds[g + 1]],
                      ds[bounds[g]:bounds[g + 1]] - g * P))

    CHt = []
    for t in range(T):
        m = 1
        for c in range(NC):
            n = len(tiles[c * T + t][0])
            m = max(m, (n + P - 1) // P)
        CHt.append(m)
    C = sum(CHt)

    per_core = []
    for c in range(NC):
        idx_flat = np.full(C * P, -1, np.int16)
        dl_cols = np.full((P, C * DLW), -1.0, np.float32)
        cnts = np.zeros((1, T), np.int32)
        off = 0
        for t in range(T):
            sl, dloc = tiles[c * T + t]
            n = len(sl)
            base = off * P
            if n == 0:
                # keep >=1 valid idx: row 0 gathered, dl -1 zeroes it
                idx_flat[base] = 0
                cnts[0, t] = 1
            else:
                cnts[0, t] = n
            if pair:
                idx_flat[base:base + n] = (sl // 2).astype(np.int16)
                par = (sl % 2).astype(np.int64)
                for q in range(2):
                    block = dl_cols[:, off * 2 + q::2][:, :CHt[t]]
                    block.T.flat[:n] = np.where(par == q, dloc, -1.0)
            else:
                idx_flat[base:base + n] = sl.astype(np.int16)
                block = dl_cols[:, off:off + CHt[t]]
                block.T.flat[:n] = dloc
            off += CHt[t]
        per_core.append({
            "idx": _wrap_idx_block(idx_flat),
            "dl": dl_cols.astype(BF),
            "cnt": cnts,
        })
    return CHt, per_core


def _dinv_cols(dinv, base, SH):
    return dinv[base:base + SH].reshape(SH // P, P).T.copy()


def _interleave(a, b):
    """[P,T],[P,T] -> [P,2T] with columns a0,b0,a1,b1,..."""
    T = a.shape[1]
    out = np.empty((P, 2 * T), np.float32)
    out[:, 0::2] = a
    out[:, 1::2] = b
    return out


# --------------------------------------------------------------------------
# device program
# --------------------------------------------------------------------------

def _build_prop(nc, tc, sb, ps, lay, prop):
    """One propagation over this core's dst shard: per tile, gather 256B
    bf16 rows, staircase one-hot, PE segment-sum in PSUM."""
    T, F, CHt = lay["T"], lay["F"], lay["CHt"]
    cnt_sb = lay["cnt_sb"]
    regs = lay["regs"]
    pair = lay["pair"]
    DLW = 2 if pair else 1
    FG = 128  # gathered row elems (bf16, 256B)
    iota = lay["iota"]
    xd = lay["table"]
    idx_t = lay["idx_t"]
    dl_t = lay["dl_t"]
    gbufs = lay["gbufs"]
    NGB = len(gbufs)

    off = 0
    for t in range(T):
        ch = CHt[t]
        gb = gbufs[t % NGB]
        reg = regs[t % len(regs)]
        nc.gpsimd.reg_load(reg, cnt_sb[0:1, t:t + 1])
        nc.gpsimd.dma_gather(
            out_ap=gb[:, : ch * FG].rearrange("p (g f) -> p g f", g=ch),
            in_ap=(xd[:].rearrange("(n two) f -> n (two f)", two=2)
                   if pair else xd[:]),
            idxs_ap=idx_t[:, off * 8:(off + ch) * 8],
            num_idxs=ch * P,
            num_idxs_reg=reg,
            elem_size=FG,
            single_packet=False,
            queue_num=t % NQ,
        )
        s_all = sb.tile([P, ch * DLW * P], BF16, tag="s_all",
                        name=f"s{lay['name']}{prop}_{t % 2}")
        nc.vector.tensor_tensor(
            out=s_all[:].rearrange("p (c j) -> p c j", c=ch * DLW),
            in0=dl_t[:, off * DLW:(off + ch) * DLW].to_broadcast([P, ch * DLW, P]),
            in1=iota[:].rearrange("p (c j) -> p c j", c=1).to_broadcast([P, ch * DLW, P]),
            op=OP.is_equal,
        )
        seg = ps.tile([P, F], F32, tag="seg")
        nmm = ch * DLW
        for d in range(ch):
            for q in range(DLW):
                k = d * DLW + q
                nc.tensor.matmul(
                    out=seg[:],
                    lhsT=s_all[:, k * P:(k + 1) * P],
                    rhs=gb[:, d * FG + q * F:d * FG + (q + 1) * F],
                    start=(k == 0),
                    stop=(k == nmm - 1),
                )
        lay["finalize"](t, seg)
        off += ch


def build_program(cfg):
    N0, N1, IN, H, OUT, NC = (cfg["N0"], cfg["N1"], cfg["IN"], cfg["H"],
                              cfg["OUT"], cfg["NC"])
    SH0, SH1 = N0 // NC, N1 // NC
    T0, T1 = SH0 // P, SH1 // P
    TP = SH0 // 2 // P
    CHt0, CHt1 = cfg["CHt0"], cfg["CHt1"]
    C0, C1 = sum(CHt0), sum(CHt1)
    GBW = max(max(CHt0), max(CHt1)) * 128  # gather buf elems (bf16)

    nc = bacc.Bacc("TRN2", target_bir_lowering=False, debug=False,
                   num_devices=NC, num_swdge_queues=NQ)

    # ---- I/O ----
    feat_sh = nc.dram_tensor("feat_sh", [SH0, IN], F32, kind="ExternalInput").ap()
    feat16_sh = nc.dram_tensor("feat16_sh", [SH0, IN], BF16, kind="ExternalInput").ap()
    dinv0_pm = nc.dram_tensor("dinv0_pm", [P, T0], F32, kind="ExternalInput").ap()
    idx0_d = nc.dram_tensor("idx0", [P, C0 * 8], I16, kind="ExternalInput").ap()
    dl0_d = nc.dram_tensor("dl0", [P, C0 * 2], BF16, kind="ExternalInput").ap()
    idx1_d = nc.dram_tensor("idx1", [P, C1 * 8], I16, kind="ExternalInput").ap()
    dl1_d = nc.dram_tensor("dl1", [P, C1], BF16, kind="ExternalInput").ap()
    nnp0 = nc.dram_tensor("nnp0", [P, 2 * T0], F32, kind="ExternalInput").ap()
    n2dv0 = nc.dram_tensor("n2dv0", [P, T0], F32, kind="ExternalInput").ap()
    pdv1s = nc.dram_tensor("pdv1s", [P, TP], F32, kind="ExternalInput").ap()
    nnp1 = nc.dram_tensor("nnp1", [P, 2 * T1], F32, kind="ExternalInput").ap()
    n2dv1 = nc.dram_tensor("n2dv1", [P, T1], F32, kind="ExternalInput").ap()
    cnt0_d = nc.dram_tensor("cnt0", [1, T0], I32, kind="ExternalInput").ap()
    cnt1_d = nc.dram_tensor("cnt1", [1, T1], I32, kind="ExternalInput").ap()
    w0_d = nc.dram_tensor("W0b", [3 * IN, H], BF16, kind="ExternalInput").ap()
    b0_d = nc.dram_tensor("b0r", [P, H], F32, kind="ExternalInput").ap()
    w1_d = nc.dram_tensor("W1b", [3 * H, H], BF16, kind="ExternalInput").ap()
    b1_d = nc.dram_tensor("b1r", [P, H], F32, kind="ExternalInput").ap()
    wc_d = nc.dram_tensor("Wc", [H, OUT], F32, kind="ExternalInput").ap()
    bc_d = nc.dram_tensor("bcr", [1, OUT], F32, kind="ExternalInput").ap()
    iota_d = nc.dram_tensor("iota16", [P, P], BF16, kind="ExternalInput").ap()
    ident_d = nc.dram_tensor("ident16", [P, P], BF16, kind="ExternalInput").ap()
    y_d = nc.dram_tensor("y", [1, OUT], F32, kind="ExternalOutput").ap()

    # ---- internal DRAM ----
    xd0_sh = nc.dram_tensor("xd0_sh", [SH0, IN], BF16).ap()
    xd0_full = nc.dram_tensor("xd0_full", [N0, IN], BF16).ap()
    x1d_sh = nc.dram_tensor("x1d_sh", [SH0, IN], BF16).ap()
    x1d_full = nc.dram_tensor("x1d_full", [N0, IN], BF16).ap()
    hbuf = nc.dram_tensor("hbuf", [SH0, H], BF16).ap()
    xpd_sh = nc.dram_tensor("xpd_sh", [SH0 // 2, H], BF16).ap()
    xpd_full = nc.dram_tensor("xpd_full", [N1, H], BF16).ap()
    x1d1_sh = nc.dram_tensor("x1d1_sh", [SH1, H], BF16).ap()
    x1d1_full = nc.dram_tensor("x1d1_full", [N1, H], BF16).ap()
    gmax_in = nc.dram_tensor("gmax_in", [P, 1], F32).ap()
    gmax_out = nc.dram_tensor("gmax_out", [P, 1], F32).ap()

    groups = [list(range(NC))]

    with tile.TileContext(nc) as tc:
        nc.gpsimd.load_library(mlp)
        with (
            tc.tile_pool(name="sb", bufs=2) as sb,
            tc.tile_pool(name="sb1", bufs=1) as sb1,
            tc.tile_pool(name="ps", bufs=2, space="PSUM") as ps,
        ):
            # ---- consts / weights ----
            iota = sb1.tile([P, P], BF16, name="iota")
            nc.sync.dma_start(iota[:], iota_d[:])
            ident = sb1.tile([P, P], BF16, name="ident")
            nc.sync.dma_start(ident[:], ident_d[:])
            w0_sb = sb1.tile([P, 2 * H], BF16, name="w0sb")
            nc.sync.dma_start(w0_sb[:, :H], w0_d[:P, :])
            nc.sync.dma_start(w0_sb[: 3 * IN - P, H:], w0_d[P:, :])
            w1_sb = sb1.tile([P, 3 * H], BF16, name="w1sb")
            for i in range(3):
                nc.sync.dma_start(w1_sb[:, i * H:(i + 1) * H], w1_d[i * P:(i + 1) * P, :])
            wc_sb = sb1.tile([P, OUT], F32, name="wcsb")
            nc.sync.dma_start(wc_sb[:], wc_d[:])
            b0_sb = sb1.tile([P, H], F32, name="b0sb")
            nc.sync.dma_start(b0_sb[:], b0_d[:])
            b1_sb = sb1.tile([P, H], F32, name="b1sb")
            nc.sync.dma_start(b1_sb[:], b1_d[:])
            bc_sb = sb1.tile([1, OUT], F32, name="bcsb")
            nc.sync.dma_start(bc_sb[:], bc_d[:])
            dvs = {}
            for nm, dr, w in (("nnp0", nnp0, 2 * T0), ("n2dv0", n2dv0, T0),
                              ("pdv1s", pdv1s, TP),
                              ("nnp1", nnp1, 2 * T1), ("n2dv1", n2dv1, T1)):
                sbt = sb1.tile([P, w], F32, name=nm + "sb")
                nc.sync.dma_start(sbt[:], dr[:])
                dvs[nm] = sbt

            # ---- persistent gather buffers (memset once: pads multiply
            # against zero staircase rows, but stale NaNs would poison) ----
            gbufs = []
            for i in range(6):
                g = sb1.tile([P, GBW], BF16, name=f"gbuf{i}")
                nc.vector.memset(g[:], 0.0)
                gbufs.append(g)

            # ---- resident idx / dl tables ----
            idx0_t = sb1.tile([P, C0 * 8], I16, name="idx0t")
            nc.sync.dma_start(idx0_t[:], idx0_d[:])
            dl0_t = sb1.tile([P, C0 * 2], BF16, name="dl0t")
            nc.sync.dma_start(dl0_t[:], dl0_d[:])
            idx1_t = sb1.tile([P, C1 * 8], I16, name="idx1t")
            nc.sync.dma_start(idx1_t[:], idx1_d[:])
            dl1_t = sb1.tile([P, C1], BF16, name="dl1t")
            nc.sync.dma_start(dl1_t[:], dl1_d[:])
            cnt0_sb = sb1.tile([1, T0], I32, name="cnt0sb")
            nc.sync.dma_start(cnt0_sb[:], cnt0_d[:])
            cnt1_sb = sb1.tile([1, T1], I32, name="cnt1sb")
            nc.sync.dma_start(cnt1_sb[:], cnt1_d[:])
            gregs = [nc.alloc_register(mybir.EngineType.Pool, f"gcnt{i}")
                     for i in range(8)]

            # ---- resident bf16 feat shard (16 tiles per big tile) ----
            NBF = min(16, T0)
            feat16_res = []
            for i in range((T0 + NBF - 1) // NBF):
                fr = sb1.tile([P, NBF * IN], BF16, name=f"f16r{i}")
                nc.sync.dma_start(
                    fr[:].rearrange("p (b f) -> p b f", b=NBF),
                    feat16_sh[:].rearrange("(b p) f -> p b f", p=P)[:, i * NBF:(i + 1) * NBF, :],
                )
                feat16_res.append(fr)

            def feat16_slice(t):
                return feat16_res[t // NBF][:, (t % NBF) * IN:(t % NBF + 1) * IN]

            # ---- persistent per-tile intermediates ----
            x1_t0 = [sb1.tile([P, 2 * IN], BF16, name=f"x1a{t}") for t in range(T0)]
            x2_t0 = [sb1.tile([P, IN], BF16, name=f"x2a{t}") for t in range(T0)]
            xp_t = [sb1.tile([P, H], BF16, name=f"xp{t}") for t in range(TP)]
            x1_t1 = [sb1.tile([P, 2 * H], BF16, name=f"x1b{t}") for t in range(T1)]
            x2_t1 = [sb1.tile([P, H], BF16, name=f"x2b{t}") for t in range(T1)]

            # ====== scale pass (own shard) + AllGather the bf16 table ======
            NB = min(16, T0)
            for g in range(0, T0, NB):
                ft = sb.tile([P, NB * IN], F32, tag="scl", name="sclf")
                nc.sync.dma_start(
                    ft[:].rearrange("p (b f) -> p b f", b=NB),
                    feat_sh[:].rearrange("(b p) f -> p b f", p=P)[:, g:g + NB, :],
                )
                dv = sb.tile([P, NB], F32, tag="scld", name="scld")
                nc.sync.dma_start(dv[:], dinv0_pm[:, g:g + NB])
                xo = sb.tile([P, NB * IN], BF16, tag="sclo", name="sclo")
                nc.vector.tensor_tensor(
                    out=xo[:].rearrange("p (b f) -> p b f", b=NB),
                    in0=ft[:].rearrange("p (b f) -> p b f", b=NB),
                    in1=dv[:].to_broadcast([P, NB, IN]),
                    op=OP.mult,
                )
                nc.sync.dma_start(
                    xd0_sh[:].rearrange("(b p) f -> p b f", p=P)[:, g:g + NB, :],
                    xo[:].rearrange("p (b f) -> p b f", b=NB),
                )
            nc.gpsimd.collective_compute(
                "AllGather", OP.bypass, replica_groups=groups,
                ins=[xd0_sh[:].opt()], outs=[xd0_full[:].opt()])

            # ================= layer 0 =================
            def fin0_p1(t, seg):
                nc.vector.tensor_tensor(
                    out=x1_t0[t][:].rearrange("p (c f) -> p c f", c=2),
                    in0=seg[:].rearrange("p (c f) -> p c f", c=1).to_broadcast([P, 2, IN]),
                    in1=dvs["nnp0"][:, 2 * t:2 * t + 2].rearrange(
                        "p (c u) -> p c u", u=1).to_broadcast([P, 2, IN]),
                    op=OP.mult)
                nc.sync.dma_start(x1d_sh[t * P:(t + 1) * P, :], x1_t0[t][:, IN:])

            lay0 = {
                "name": "l0", "T": T0, "F": IN, "CHt": CHt0, "pair": True,
                "table": xd0_full, "idx_t": idx0_t, "dl_t": dl0_t,
                "cnt_sb": cnt0_sb, "regs": gregs,
                "gbufs": gbufs, "iota": iota, "finalize": fin0_p1,
            }
            _build_prop(nc, tc, sb, ps, lay0, 1)

            nc.gpsimd.collective_compute(
                "AllGather", OP.bypass, replica_groups=groups,
                ins=[x1d_sh[:].opt()], outs=[x1d_full[:].opt()])

            def fin0_p2(t, seg):
                nc.vector.scalar_tensor_tensor(
                    out=x2_t0[t][:], in0=seg[:],
                    scalar=dvs["n2dv0"][:, t:t + 1], in1=feat16_slice(t),
                    op0=OP.mult, op1=OP.subtract)

            lay0p2 = dict(lay0)
            lay0p2["table"] = x1d_full
            lay0p2["finalize"] = fin0_p2
            _build_prop(nc, tc, sb, ps, lay0p2, 2)

            # ---- layer-0 dense: H = relu([X0|X1|X2] @ W0 + b0) ----
            for t in range(T0):
                trA = ps.tile([P, P], BF16, tag="trA")
                nc.tensor.transpose(out=trA[:IN, :], in_=feat16_slice(t), identity=ident[:])
                nc.tensor.transpose(out=trA[IN:2 * IN, :], in_=x1_t0[t][:, :IN], identity=ident[:])
                xcatA = sb.tile([P, P], BF16, tag="xcatA", name="xcatA")
                nc.scalar.activation(xcatA[:], trA[:], AF.Copy)
                trB = ps.tile([P, P], BF16, tag="trA")
                nc.tensor.transpose(out=trB[:IN, :], in_=x2_t0[t][:], identity=ident[:])
                xcatB = sb.tile([P, P], BF16, tag="xcatB", name="xcatB")
                nc.scalar.activation(xcatB[:IN, :], trB[:IN, :], AF.Copy)

                hps = ps.tile([P, H], F32, tag="hps")
                nc.tensor.matmul(out=hps[:], lhsT=xcatA[:],
                                 rhs=w0_sb[:, :H], start=True, stop=False)
                nc.tensor.matmul(out=hps[:], lhsT=xcatB[:IN, :],
                                 rhs=w0_sb[:IN, H:2 * H], start=False, stop=True)
                hsb = sb.tile([P, H], BF16, tag="hsb", name="hsb")
                nc.vector.tensor_tensor(out=hsb[:], in0=hps[:],
                                        in1=b0_sb[:], op=OP.add)
                nc.scalar.activation(hsb[:], hsb[:], AF.Relu)
                nc.sync.dma_start(hbuf[t * P:(t + 1) * P, :], hsb[:])

            # ---- pooling + scale for layer 1 ----
            for t in range(TP):
                ev = sb.tile([P, H], BF16, tag="pev", name="pev")
                nc.sync.dma_start(
                    ev[:], hbuf[:].rearrange("(n two) h -> n two h", two=2)[t * P:(t + 1) * P, 0, :])
                od = sb.tile([P, H], BF16, tag="pod", name="pod")
                nc.sync.dma_start(
                    od[:], hbuf[:].rearrange("(n two) h -> n two h", two=2)[t * P:(t + 1) * P, 1, :])
                nc.vector.tensor_tensor(out=xp_t[t][:], in0=ev[:], in1=od[:], op=OP.max)
                xpd = sb.tile([P, H], BF16, tag="pxd", name="pxd")
                nc.vector.tensor_tensor(
                    out=xpd[:], in0=xp_t[t][:],
                    in1=dvs["pdv1s"][:, t:t + 1].to_broadcast([P, H]), op=OP.mult)
                nc.sync.dma_start(xpd_sh[t * P:(t + 1) * P, :], xpd[:])

            nc.gpsimd.collective_compute(
                "AllGather", OP.bypass, replica_groups=groups,
                ins=[xpd_sh[:].opt()], outs=[xpd_full[:].opt()])

            # ================= layer 1 =================
            def fin1_p1(t, seg):
                nc.vector.tensor_tensor(
                    out=x1_t1[t][:].rearrange("p (c f) -> p c f", c=2),
                    in0=seg[:].rearrange("p (c f) -> p c f", c=1).to_broadcast([P, 2, H]),
                    in1=dvs["nnp1"][:, 2 * t:2 * t + 2].rearrange(
                        "p (c u) -> p c u", u=1).to_broadcast([P, 2, H]),
                    op=OP.mult)
                nc.sync.dma_start(x1d1_sh[t * P:(t + 1) * P, :], x1_t1[t][:, H:])

            lay1 = {
                "name": "l1", "T": T1, "F": H, "CHt": CHt1, "pair": False,
                "table": xpd_full, "idx_t": idx1_t, "dl_t": dl1_t,
                "cnt_sb": cnt1_sb, "regs": gregs,
                "gbufs": gbufs, "iota": iota, "finalize": fin1_p1,
            }
            _build_prop(nc, tc, sb, ps, lay1, 1)

            nc.gpsimd.collective_compute(
                "AllGather", OP.bypass, replica_groups=groups,
                ins=[x1d1_sh[:].opt()], outs=[x1d1_full[:].opt()])

            def fin1_p2(t, seg):
                nc.vector.scalar_tensor_tensor(
                    out=x2_t1[t][:], in0=seg[:],
                    scalar=dvs["n2dv1"][:, t:t + 1], in1=xp_t[t][:],
                    op0=OP.mult, op1=OP.subtract)

            lay1p2 = dict(lay1)
            lay1p2["table"] = x1d1_full
            lay1p2["finalize"] = fin1_p2
            _build_prop(nc, tc, sb, ps, lay1p2, 2)

            # ---- layer-1 dense + global max ----
            gmax = sb1.tile([P, 1], F32, name="gmax")
            nc.vector.memset(gmax[:], -3.0e38)
            for t in range(T1):
                hps = ps.tile([P, H], F32, tag="hps")
                for i, xt in enumerate([xp_t[t][:], x1_t1[t][:, :H], x2_t1[t][:]]):
                    tr = ps.tile([P, P], BF16, tag="trA")
                    nc.tensor.transpose(out=tr[:], in_=xt, identity=ident[:])
                    xT = sb.tile([P, P], BF16, tag="xcatA", name=f"m1T{i}")
                    nc.scalar.activation(xT[:], tr[:], AF.Copy)
                    nc.tensor.matmul(out=hps[:], lhsT=xT[:],
                                     rhs=w1_sb[:, i * H:(i + 1) * H],
                                     start=(i == 0), stop=(i == 2))
                hsb = sb.tile([P, H], BF16, tag="hsb", name="m1h")
                nc.vector.tensor_tensor(out=hsb[:], in0=hps[:],
                                        in1=b1_sb[:], op=OP.add)
                nc.scalar.activation(hsb[:], hsb[:], AF.Relu)
                tr = ps.tile([P, P], BF16, tag="trA")
                nc.tensor.transpose(out=tr[:], in_=hsb[:], identity=ident[:])
                tmax = sb.tile([P, 1], F32, tag="tmax", name="m1t")
                nc.vector.tensor_reduce(out=tmax[:], in_=tr[:], axis=AX.X, op=OP.max)
                nc.vector.tensor_tensor(out=gmax[:], in0=gmax[:], in1=tmax[:], op=OP.max)

            nc.sync.dma_start(gmax_in[:], gmax[:])
            nc.gpsimd.collective_compute(
                "AllReduce", OP.max, replica_groups=groups,
                ins=[gmax_in[:].opt()], outs=[gmax_out[:].opt()])
            gmax2 = sb1.tile([P, 1], F32, name="gmax2")
            nc.sync.dma_start(gmax2[:], gmax_out[:])

            zps = ps.tile([1, OUT], F32, tag="seg")
            nc.tensor.matmul(out=zps[:], lhsT=gmax2[:], rhs=wc_sb[:, :OUT],
                             start=True, stop=True)
            z = sb1.tile([1, OUT], F32, name="zrow")
            nc.vector.tensor_tensor(out=z[:], in0=zps[:], in1=bc_sb[:], op=OP.add)
            m = sb1.tile([1, 1], F32, name="mrow")
            nc.vector.tensor_reduce(out=m[:], in_=z[:], axis=AX.X, op=OP.max)
            zc = sb1.tile([1, OUT], F32, name="zcrow")
            nc.vector.tensor_tensor(out=zc[:], in0=z[:],
                                    in1=m[:].to_broadcast([1, OUT]), op=OP.subtract)
            ez = sb1.tile([1, OUT], F32, name="ezrow")
            nc.scalar.activation(ez[:], zc[:], AF.Exp)
            s = sb1.tile([1, 1], F32, name="srow")
            nc.vector.tensor_reduce(out=s[:], in_=ez[:], axis=AX.X, op=OP.add)
            ls = sb1.tile([1, 1], F32, name="lsrow")
            nc.scalar.activation(ls[:], s[:], AF.Ln)
            yv = sb1.tile([1, OUT], F32, name="yrow")
            nc.vector.tensor_tensor(out=yv[:], in0=zc[:],
                                    in1=ls[:].to_broadcast([1, OUT]), op=OP.subtract)
            nc.sync.dma_start(y_d[:], yv[:])

    nc.compile()
    return nc


# --------------------------------------------------------------------------
# host entry
# --------------------------------------------------------------------------

_CACHE = {}


def prepare(feat, src0, dst0, src1, dst1, W0, b0, W1, b1, Wc, bc, NC=8):
    N0, IN = feat.shape
    N1 = N0 // 2
    H = W0.shape[1]
    OUT = Wc.shape[1]
    SH0, SH1 = N0 // NC, N1 // NC
    T0, T1, TP = SH0 // P, SH1 // P, SH0 // 2 // P

    feat = np.asarray(feat, np.float32)
    src0 = np.asarray(src0)
    dst0 = np.asarray(dst0)
    src1 = np.asarray(src1)
    dst1 = np.asarray(dst1)

    CHt0, pc0 = _prep_layer(src0, dst0, N0, NC, True)
    CHt1, pc1 = _prep_layer(src1, dst1, N1, NC, False)

    deg0 = np.bincount(dst0, minlength=N0).astype(np.float32)
    dinv0 = 1.0 / np.sqrt(np.maximum(deg0, 1.0))
    deg1 = np.bincount(dst1, minlength=N1).astype(np.float32)
    dinv1 = 1.0 / np.sqrt(np.maximum(deg1, 1.0))

    key = (N0, IN, H, OUT, NC, tuple(CHt0), tuple(CHt1))
    if key not in _CACHE:
        cfg = {"N0": N0, "N1": N1, "IN": IN, "H": H, "OUT": OUT, "NC": NC,
               "CHt0": CHt0, "CHt1": CHt1}
        _CACHE[key] = build_program(cfg)
    nc = _CACHE[key]

    iota_np = np.broadcast_to(np.arange(P, dtype=np.float32), (P, P)).astype(BF)
    ident_np = np.eye(P, dtype=np.float32).astype(BF)

    in_maps = []
    for c in range(NC):
        m = {
            "feat_sh": feat[c * SH0:(c + 1) * SH0],
            "feat16_sh": feat[c * SH0:(c + 1) * SH0].astype(BF),
            "dinv0_pm": _dinv_cols(dinv0, c * SH0, SH0),
            "idx0": pc0[c]["idx"],
            "dl0": pc0[c]["dl"],
            "cnt0": pc0[c]["cnt"],
            "idx1": pc1[c]["idx"],
            "dl1": pc1[c]["dl"],
            "cnt1": pc1[c]["cnt"],
            "nnp0": _interleave(-_dinv_cols(dinv0, c * SH0, SH0),
                                -(_dinv_cols(dinv0, c * SH0, SH0) ** 2)),
            "n2dv0": -2.0 * _dinv_cols(dinv0, c * SH0, SH0),
            "pdv1s": _dinv_cols(dinv1, c * SH0 // 2, SH0 // 2),
            "nnp1": _interleave(-_dinv_cols(dinv1, c * SH1, SH1),
                                -(_dinv_cols(dinv1, c * SH1, SH1) ** 2)),
            "n2dv1": -2.0 * _dinv_cols(dinv1, c * SH1, SH1),
            "W0b": np.asarray(W0, np.float32).astype(BF),
            "b0r": np.broadcast_to(np.asarray(b0, np.float32), (P, H)).copy(),
            "W1b": np.asarray(W1, np.float32).astype(BF),
            "b1r": np.broadcast_to(np.asarray(b1, np.float32), (P, H)).copy(),
            "Wc": np.asarray(Wc, np.float32),
            "bcr": np.asarray(bc, np.float32).reshape(1, OUT),
            "iota16": iota_np,
            "ident16": ident_np,
        }
        in_maps.append(m)

    return nc, in_maps


def run(feat, src0, dst0, src1, dst1, W0, b0, W1, b1, Wc, bc, NC=8, **rkw):
    nc, in_maps = prepare(feat, src0, dst0, src1, dst1, W0, b0, W1, b1, Wc, bc, NC)
    res = bass_utils.run_bass_kernel_spmd(nc, in_maps, core_ids=list(range(NC)), **rkw)
    return res.results[0]["y"], res


def kernel(**inputs):
    y, _ = run(**inputs)
    return y


# revision 18
# speedup vs baseline: 1.0087x; 1.0087x over previous
"""ChebNet (K=3, 2 ChebConv layers + node-halve maxpool + global max + FC +
log_softmax) on 8 Trainium2 NeuronCores — v2.

vs v1:
- bf16 gather tables: 256B rows both layers (halves gather DMA bytes)
- trailing -1 index padding: Q7 SWDGE trims pads, desc-gen ~ actual edges
- per-tile-slot static chunk counts (max over the 8 cores) instead of a
  global max -> less padding everywhere
- 4 SWDGE queues round-robin + 6 rotating gather buffers (parallel desc gen)
- bf16 staircase / matmuls / dense / collectives; f32 PSUM accumulation
- x1/x2/xp/l1 intermediates SBUF-resident (no DRAM round trips)
"""

import numpy as np
import ml_dtypes

import concourse.bass as bass
import concourse.bacc as bacc
import concourse.mybir as mybir
import concourse.tile as tile
import concourse.bass_utils as bass_utils
from concourse.library_config import mlp

F32 = mybir.dt.float32
BF16 = mybir.dt.bfloat16
I16 = mybir.dt.int16
I32 = mybir.dt.int32
AX = mybir.AxisListType
OP = mybir.AluOpType
AF = mybir.ActivationFunctionType
BF = ml_dtypes.bfloat16

P = 128
NQ = 4  # SWDGE queues


# --------------------------------------------------------------------------
# host-side graph preprocessing
# --------------------------------------------------------------------------

def _wrap_idx_block(vals):
    """Wrap flat int16 idx list (len % 128 == 0) into dma_gather SBUF layout:
    index i -> partition i % 16, column i // 16, replicated over 8 groups."""
    n = len(vals)
    blk = vals.reshape(n // 16, 16).T
    return np.tile(blk, (8, 1))


def _prep_layer(src, dst, N, NC, pair):
    """Sort edges by dst, bucket into 128-dst tiles, chunk by 128 edges.

    Chunk counts per tile-slot are the max over the NC cores (one SPMD
    program), but each core pads its own tail with idx -1 (Q7-trimmed).

    Returns (CHt, per_core): CHt[t] = chunks for tile-slot t;
    per_core[c] = {idx: [128, C*8] i16, dl: [128, C*DLW] bf16} with
    C = sum(CHt), DLW = 2 if pair else 1.
    """
    SH = N // NC
    T = SH // P
    DLW = 2 if pair else 1
    order = np.argsort(dst, kind="stable")
    ds = dst[order]
    ss = src[order].astype(np.int64)
    bounds = np.searchsorted(ds, np.arange(0, N + 1, P))
    tiles = []
    for g in range(N // P):
        tiles.append((ss[bounds[g]:bounds[g + 1]],
                      ds[bounds[g]:bounds[g + 1]] - g * P))

    CHt = []
    for t in range(T):
        m = 1
        for c in range(NC):
            n = len(tiles[c * T + t][0])
            m = max(m, (n + P - 1) // P)
        CHt.append(m)
    C = sum(CHt)

    per_core = []
    for c in range(NC):
        idx_flat = np.full(C * P, -1, np.int16)
        dl_cols = np.full((P, C * DLW), -1.0, np.float32)
        cnts = np.zeros((1, T), np.int32)
        off = 0
        for t in range(T):
            sl, dloc = tiles[c * T + t]
            n = len(sl)
            base = off * P
            if n == 0:
                # keep >=1 valid idx: row 0 gathered, dl -1 zeroes it
                idx_flat[base] = 0
                cnts[0, t] = 1
            else:
                cnts[0, t] = n
            if pair:
                idx_flat[base:base + n] = (sl // 2).astype(np.int16)
                par = (sl % 2).astype(np.int64)
                for q in range(2):
                    block = dl_cols[:, off * 2 + q::2][:, :CHt[t]]
                    block.T.flat[:n] = np.where(par == q, dloc, -1.0)
            else:
                idx_flat[base:base + n] = sl.astype(np.int16)
                block = dl_cols[:, off:off + CHt[t]]
                block.T.flat[:n] = dloc
            off += CHt[t]
        per_core.append({
            "idx": _wrap_idx_block(idx_flat),
            "dl": dl_cols.astype(BF),
            "cnt": cnts,
        })
    return CHt, per_core


def _dinv_cols(dinv, base, SH):
    return dinv[base:base + SH].reshape(SH // P, P).T.copy()


def _interleave(a, b):
    """[P,T],[P,T] -> [P,2T] with columns a0,b0,a1,b1,..."""
    T = a.shape[1]
    out = np.empty((P, 2 * T), np.float32)
    out[:, 0::2] = a
    out[:, 1::2] = b
    return out


# --------------------------------------------------------------------------
# device program
# --------------------------------------------------------------------------

def _build_prop(nc, tc, sb, ps, lay, prop):
    """One propagation over this core's dst shard: per tile, gather 256B
    bf16 rows, staircase one-hot, PE segment-sum in PSUM."""
    T, F, CHt = lay["T"], lay["F"], lay["CHt"]
    cnt_sb = lay["cnt_sb"]
    regs = lay["regs"]
    pair = lay["pair"]
    DLW = 2 if pair else 1
    FG = 128  # gathered row elems (bf16, 256B)
    iota = lay["iota"]
    xd = lay["table"]
    idx_t = lay["idx_t"]
    dl_t = lay["dl_t"]
    gbufs = lay["gbufs"]
    NGB = len(gbufs)

    off = 0
    for t in range(T):
        ch = CHt[t]
        gb = gbufs[t % NGB]
        reg = regs[t % len(regs)]
        nc.gpsimd.reg_load(reg, cnt_sb[0:1, t:t + 1])
        nc.gpsimd.dma_gather(
            out_ap=gb[:, : ch * FG].rearrange("p (g f) -> p g f", g=ch),
            in_ap=(xd[:].rearrange("(n two) f -> n (two f)", two=2)
                   if pair else xd[:]),
            idxs_ap=idx_t[:, off * 8:(off + ch) * 8],
            num_idxs=ch * P,
            num_idxs_reg=reg,
            elem_size=FG,
            single_packet=False,
            queue_num=t % NQ,
        )
        s_all = sb.tile([P, ch * DLW * P], BF16, tag="s_all",
                        name=f"s{lay['name']}{prop}_{t % 2}")
        nc.vector.tensor_tensor(
            out=s_all[:].rearrange("p (c j) -> p c j", c=ch * DLW),
            in0=dl_t[:, off * DLW:(off + ch) * DLW].to_broadcast([P, ch * DLW, P]),
            in1=iota[:].rearrange("p (c j) -> p c j", c=1).to_broadcast([P, ch * DLW, P]),
            op=OP.is_equal,
        )
        seg = ps.tile([P, F], F32, tag="seg")
        nmm = ch * DLW
        for d in range(ch):
            for q in range(DLW):
                k = d * DLW + q
                nc.tensor.matmul(
                    out=seg[:],
                    lhsT=s_all[:, k * P:(k + 1) * P],
                    rhs=gb[:, d * FG + q * F:d * FG + (q + 1) * F],
                    start=(k == 0),
                    stop=(k == nmm - 1),
                )
        lay["finalize"](t, seg)
        off += ch


def build_program(cfg):
    N0, N1, IN, H, OUT, NC = (cfg["N0"], cfg["N1"], cfg["IN"], cfg["H"],
                              cfg["OUT"], cfg["NC"])
    SH0, SH1 = N0 // NC, N1 // NC
    T0, T1 = SH0 // P, SH1 // P
    TP = SH0 // 2 // P
    CHt0, CHt1 = cfg["CHt0"], cfg["CHt1"]
    C0, C1 = sum(CHt0), sum(CHt1)
    GBW = max(max(CHt0), max(CHt1)) * 128  # gather buf elems (bf16)

    nc = bacc.Bacc("TRN2", target_bir_lowering=False, debug=False,
                   num_devices=NC, num_swdge_queues=NQ)

    # ---- I/O ----
    feat_sh = nc.dram_tensor("feat_sh", [SH0, IN], F32, kind="ExternalInput").ap()
    feat16_sh = nc.dram_tensor("feat16_sh", [SH0, IN], BF16, kind="ExternalInput").ap()
    dinv0_pm = nc.dram_tensor("dinv0_pm", [P, T0], F32, kind="ExternalInput").ap()
    idx0_d = nc.dram_tensor("idx0", [P, C0 * 8], I16, kind="ExternalInput").ap()
    dl0_d = nc.dram_tensor("dl0", [P, C0 * 2], BF16, kind="ExternalInput").ap()
    idx1_d = nc.dram_tensor("idx1", [P, C1 * 8], I16, kind="ExternalInput").ap()
    dl1_d = nc.dram_tensor("dl1", [P, C1], BF16, kind="ExternalInput").ap()
    nnp0 = nc.dram_tensor("nnp0", [P, 2 * T0], F32, kind="ExternalInput").ap()
    n2dv0 = nc.dram_tensor("n2dv0", [P, T0], F32, kind="ExternalInput").ap()
    pdv1s = nc.dram_tensor("pdv1s", [P, TP], F32, kind="ExternalInput").ap()
    nnp1 = nc.dram_tensor("nnp1", [P, 2 * T1], F32, kind="ExternalInput").ap()
    n2dv1 = nc.dram_tensor("n2dv1", [P, T1], F32, kind="ExternalInput").ap()
    cnt0_d = nc.dram_tensor("cnt0", [1, T0], I32, kind="ExternalInput").ap()
    cnt1_d = nc.dram_tensor("cnt1", [1, T1], I32, kind="ExternalInput").ap()
    w0_d = nc.dram_tensor("W0b", [3 * IN, H], BF16, kind="ExternalInput").ap()
    b0_d = nc.dram_tensor("b0r", [P, H], F32, kind="ExternalInput").ap()
    w1_d = nc.dram_tensor("W1b", [3 * H, H], BF16, kind="ExternalInput").ap()
    b1_d = nc.dram_tensor("b1r", [P, H], F32, kind="ExternalInput").ap()
    wc_d = nc.dram_tensor("Wc", [H, OUT], F32, kind="ExternalInput").ap()
    bc_d = nc.dram_tensor("bcr", [1, OUT], F32, kind="ExternalInput").ap()
    iota_d = nc.dram_tensor("iota16", [P, P], BF16, kind="ExternalInput").ap()
    ident_d = nc.dram_tensor("ident16", [P, P], BF16, kind="ExternalInput").ap()
    y_d = nc.dram_tensor("y", [1, OUT], F32, kind="ExternalOutput").ap()

    # ---- internal DRAM ----
    xd0_sh = nc.dram_tensor("xd0_sh", [SH0, IN], BF16).ap()
    xd0_full = nc.dram_tensor("xd0_full", [N0, IN], BF16).ap()
    x1d_sh = nc.dram_tensor("x1d_sh", [SH0, IN], BF16).ap()
    x1d_full = nc.dram_tensor("x1d_full", [N0, IN], BF16).ap()
    hbuf = nc.dram_tensor("hbuf", [SH0, H], BF16).ap()
    xpd_sh = nc.dram_tensor("xpd_sh", [SH0 // 2, H], BF16).ap()
    xpd_full = nc.dram_tensor("xpd_full", [N1, H], BF16).ap()
    x1d1_sh = nc.dram_tensor("x1d1_sh", [SH1, H], BF16).ap()
    x1d1_full = nc.dram_tensor("x1d1_full", [N1, H], BF16).ap()
    gmax_in = nc.dram_tensor("gmax_in", [P, 1], F32).ap()
    gmax_out = nc.dram_tensor("gmax_out", [P, 1], F32).ap()

    groups = [list(range(NC))]

    with tile.TileContext(nc) as tc:
        nc.gpsimd.load_library(mlp)
        with (
            tc.tile_pool(name="sb", bufs=2) as sb,
            tc.tile_pool(name="sb1", bufs=1) as sb1,
            tc.tile_pool(name="ps", bufs=2, space="PSUM") as ps,
        ):
            # ---- consts / weights ----
            iota = sb1.tile([P, P], BF16, name="iota")
            nc.sync.dma_start(iota[:], iota_d[:])
            ident = sb1.tile([P, P], BF16, name="ident")
            nc.sync.dma_start(ident[:], ident_d[:])
            w0_sb = sb1.tile([P, 2 * H], BF16, name="w0sb")
            nc.sync.dma_start(w0_sb[:, :H], w0_d[:P, :])
            nc.sync.dma_start(w0_sb[: 3 * IN - P, H:], w0_d[P:, :])
            w1_sb = sb1.tile([P, 3 * H], BF16, name="w1sb")
            for i in range(3):
                nc.sync.dma_start(w1_sb[:, i * H:(i + 1) * H], w1_d[i * P:(i + 1) * P, :])
            wc_sb = sb1.tile([P, OUT], F32, name="wcsb")
            nc.sync.dma_start(wc_sb[:], wc_d[:])
            b0_sb = sb1.tile([P, H], F32, name="b0sb")
            nc.sync.dma_start(b0_sb[:], b0_d[:])
            b1_sb = sb1.tile([P, H], F32, name="b1sb")
            nc.sync.dma_start(b1_sb[:], b1_d[:])
            bc_sb = sb1.tile([1, OUT], F32, name="bcsb")
            nc.sync.dma_start(bc_sb[:], bc_d[:])
            dvs = {}
            for nm, dr, w in (("nnp0", nnp0, 2 * T0), ("n2dv0", n2dv0, T0),
                              ("pdv1s", pdv1s, TP),
                              ("nnp1", nnp1, 2 * T1), ("n2dv1", n2dv1, T1)):
                sbt = sb1.tile([P, w], F32, name=nm + "sb")
                nc.sync.dma_start(sbt[:], dr[:])
                dvs[nm] = sbt

            # ---- persistent gather buffers (memset once: pads multiply
            # against zero staircase rows, but stale NaNs would poison) ----
            gbufs = []
            for i in range(6):
                g = sb1.tile([P, GBW], BF16, name=f"gbuf{i}")
                nc.vector.memset(g[:], 0.0)
                gbufs.append(g)

            # ---- resident idx / dl tables ----
            idx0_t = sb1.tile([P, C0 * 8], I16, name="idx0t")
            nc.sync.dma_start(idx0_t[:], idx0_d[:])
            dl0_t = sb1.tile([P, C0 * 2], BF16, name="dl0t")
            nc.sync.dma_start(dl0_t[:], dl0_d[:])
            idx1_t = sb1.tile([P, C1 * 8], I16, name="idx1t")
            nc.sync.dma_start(idx1_t[:], idx1_d[:])
            dl1_t = sb1.tile([P, C1], BF16, name="dl1t")
            nc.sync.dma_start(dl1_t[:], dl1_d[:])
            cnt0_sb = sb1.tile([1, T0], I32, name="cnt0sb")
            nc.sync.dma_start(cnt0_sb[:], cnt0_d[:])
            cnt1_sb = sb1.tile([1, T1], I32, name="cnt1sb")
            nc.sync.dma_start(cnt1_sb[:], cnt1_d[:])
            gregs = [nc.alloc_register(mybir.EngineType.Pool, f"gcnt{i}")
                     for i in range(8)]

            # ---- resident bf16 feat shard (16 tiles per big tile) ----
            NBF = min(16, T0)
            feat16_res = []
            for i in range((T0 + NBF - 1) // NBF):
                fr = sb1.tile([P, NBF * IN], BF16, name=f"f16r{i}")
                nc.sync.dma_start(
                    fr[:].rearrange("p (b f) -> p b f", b=NBF),
                    feat16_sh[:].rearrange("(b p) f -> p b f", p=P)[:, i * NBF:(i + 1) * NBF, :],
                )
                feat16_res.append(fr)

            def feat16_slice(t):
                return feat16_res[t // NBF][:, (t % NBF) * IN:(t % NBF + 1) * IN]

            # ---- persistent per-tile intermediates ----
            x1_t0 = [sb1.tile([P, 2 * IN], BF16, name=f"x1a{t}") for t in range(T0)]
            xcatA_t = [sb1.tile([P, P], BF16, name=f"xcA{t}") for t in range(T0)]
            x2_t0 = [sb1.tile([P, IN], BF16, name=f"x2a{t}") for t in range(T0)]
            xp_t = [sb1.tile([P, H], BF16, name=f"xp{t}") for t in range(TP)]
            x1_t1 = [sb1.tile([P, 2 * H], BF16, name=f"x1b{t}") for t in range(T1)]
            x2_t1 = [sb1.tile([P, H], BF16, name=f"x2b{t}") for t in range(T1)]

            # ====== scale pass (own shard) + AllGather the bf16 table ======
            NB = min(16, T0)
            for g in range(0, T0, NB):
                ft = sb.tile([P, NB * IN], F32, tag="scl", name="sclf")
                nc.sync.dma_start(
                    ft[:].rearrange("p (b f) -> p b f", b=NB),
                    feat_sh[:].rearrange("(b p) f -> p b f", p=P)[:, g:g + NB, :],
                )
                dv = sb.tile([P, NB], F32, tag="scld", name="scld")
                nc.sync.dma_start(dv[:], dinv0_pm[:, g:g + NB])
                xo = sb.tile([P, NB * IN], BF16, tag="sclo", name="sclo")
                nc.vector.tensor_tensor(
                    out=xo[:].rearrange("p (b f) -> p b f", b=NB),
                    in0=ft[:].rearrange("p (b f) -> p b f", b=NB),
                    in1=dv[:].to_broadcast([P, NB, IN]),
                    op=OP.mult,
                )
                nc.sync.dma_start(
                    xd0_sh[:].rearrange("(b p) f -> p b f", p=P)[:, g:g + NB, :],
                    xo[:].rearrange("p (b f) -> p b f", b=NB),
                )
            nc.gpsimd.collective_compute(
                "AllGather", OP.bypass, replica_groups=groups,
                ins=[xd0_sh[:].opt()], outs=[xd0_full[:].opt()])

            # ================= layer 0 =================
            def fin0_p1(t, seg):
                nc.vector.tensor_tensor(
                    out=x1_t0[t][:].rearrange("p (c f) -> p c f", c=2),
                    in0=seg[:].rearrange("p (c f) -> p c f", c=1).to_broadcast([P, 2, IN]),
                    in1=dvs["nnp0"][:, 2 * t:2 * t + 2].rearrange(
                        "p (c u) -> p c u", u=1).to_broadcast([P, 2, IN]),
                    op=OP.mult)
                nc.sync.dma_start(x1d_sh[t * P:(t + 1) * P, :], x1_t0[t][:, IN:])

            lay0 = {
                "name": "l0", "T": T0, "F": IN, "CHt": CHt0, "pair": True,
                "table": xd0_full, "idx_t": idx0_t, "dl_t": dl0_t,
                "cnt_sb": cnt0_sb, "regs": gregs,
                "gbufs": gbufs, "iota": iota, "finalize": fin0_p1,
            }
            _build_prop(nc, tc, sb, ps, lay0, 1)

            for t in range(T0):
                trA = ps.tile([P, P], BF16, tag="trA")
                nc.tensor.transpose(out=trA[:IN, :], in_=feat16_slice(t), identity=ident[:])
                nc.tensor.transpose(out=trA[IN:2 * IN, :], in_=x1_t0[t][:, :IN], identity=ident[:])
                nc.vector.tensor_copy(xcatA_t[t][:], trA[:])

            nc.gpsimd.collective_compute(
                "AllGather", OP.bypass, replica_groups=groups,
                ins=[x1d_sh[:].opt()], outs=[x1d_full[:].opt()])

            def fin0_p2(t, seg):
                x2a = sb.tile([P, IN], BF16, tag="fin", name="f0c")
                nc.vector.tensor_tensor(
                    out=x2a[:], in0=seg[:],
                    in1=dvs["n2dv0"][:, t:t + 1].to_broadcast([P, IN]), op=OP.mult)
                nc.vector.tensor_tensor(
                    out=x2_t0[t][:], in0=x2a[:], in1=feat16_slice(t), op=OP.subtract)

            lay0p2 = dict(lay0)
            lay0p2["table"] = x1d_full
            lay0p2["finalize"] = fin0_p2
            _build_prop(nc, tc, sb, ps, lay0p2, 2)

            # ---- layer-0 dense: H = relu([X0|X1|X2] @ W0 + b0) ----
            for t in range(T0):
                trB = ps.tile([P, P], BF16, tag="trA")
                nc.tensor.transpose(out=trB[:IN, :], in_=x2_t0[t][:], identity=ident[:])
                xcatB = sb.tile([P, P], BF16, tag="xcatB", name="xcatB")
                nc.vector.tensor_copy(xcatB[:IN, :], trB[:IN, :])

                hps = ps.tile([P, H], F32, tag="hps")
                nc.tensor.matmul(out=hps[:], lhsT=xcatA_t[t][:],
                                 rhs=w0_sb[:, :H], start=True, stop=False)
                nc.tensor.matmul(out=hps[:], lhsT=xcatB[:IN, :],
                                 rhs=w0_sb[:IN, H:2 * H], start=False, stop=True)
                hsb = sb.tile([P, H], BF16, tag="hsb", name="hsb")
                nc.vector.tensor_tensor(out=hsb[:], in0=hps[:],
                                        in1=b0_sb[:], op=OP.add)
                nc.vector.tensor_scalar_max(hsb[:], hsb[:], 0.0)
                nc.sync.dma_start(hbuf[t * P:(t + 1) * P, :], hsb[:])

            # ---- pooling + scale for layer 1 ----
            for t in range(TP):
                ev = sb.tile([P, H], BF16, tag="pev", name="pev")
                nc.sync.dma_start(
                    ev[:], hbuf[:].rearrange("(n two) h -> n two h", two=2)[t * P:(t + 1) * P, 0, :])
                od = sb.tile([P, H], BF16, tag="pod", name="pod")
                nc.sync.dma_start(
                    od[:], hbuf[:].rearrange("(n two) h -> n two h", two=2)[t * P:(t + 1) * P, 1, :])
                nc.vector.tensor_tensor(out=xp_t[t][:], in0=ev[:], in1=od[:], op=OP.max)
                xpd = sb.tile([P, H], BF16, tag="pxd", name="pxd")
                nc.vector.tensor_tensor(
                    out=xpd[:], in0=xp_t[t][:],
                    in1=dvs["pdv1s"][:, t:t + 1].to_broadcast([P, H]), op=OP.mult)
                nc.sync.dma_start(xpd_sh[t * P:(t + 1) * P, :], xpd[:])

            nc.gpsimd.collective_compute(
                "AllGather", OP.bypass, replica_groups=groups,
                ins=[xpd_sh[:].opt()], outs=[xpd_full[:].opt()])

            # ================= layer 1 =================
            def fin1_p1(t, seg):
                nc.vector.tensor_tensor(
                    out=x1_t1[t][:].rearrange("p (c f) -> p c f", c=2),
                    in0=seg[:].rearrange("p (c f) -> p c f", c=1).to_broadcast([P, 2, H]),
                    in1=dvs["nnp1"][:, 2 * t:2 * t + 2].rearrange(
                        "p (c u) -> p c u", u=1).to_broadcast([P, 2, H]),
                    op=OP.mult)
                nc.sync.dma_start(x1d1_sh[t * P:(t + 1) * P, :], x1_t1[t][:, H:])

            lay1 = {
                "name": "l1", "T": T1, "F": H, "CHt": CHt1, "pair": False,
                "table": xpd_full, "idx_t": idx1_t, "dl_t": dl1_t,
                "cnt_sb": cnt1_sb, "regs": gregs,
                "gbufs": gbufs, "iota": iota, "finalize": fin1_p1,
            }
            _build_prop(nc, tc, sb, ps, lay1, 1)

            nc.gpsimd.collective_compute(
                "AllGather", OP.bypass, replica_groups=groups,
                ins=[x1d1_sh[:].opt()], outs=[x1d1_full[:].opt()])

            def fin1_p2(t, seg):
                x2a = sb.tile([P, H], BF16, tag="fin", name="f1c")
                nc.vector.tensor_tensor(
                    out=x2a[:], in0=seg[:],
                    in1=dvs["n2dv1"][:, t:t + 1].to_broadcast([P, H]), op=OP.mult)
                nc.vector.tensor_tensor(
                    out=x2_t1[t][:], in0=x2a[:], in1=xp_t[t][:], op=OP.subtract)

            lay1p2 = dict(lay1)
            lay1p2["table"] = x1d1_full
            lay1p2["finalize"] = fin1_p2
            _build_prop(nc, tc, sb, ps, lay1p2, 2)

            # ---- layer-1 dense + global max ----
            gmax = sb1.tile([P, 1], F32, name="gmax")
            nc.vector.memset(gmax[:], -3.0e38)
            for t in range(T1):
                hps = ps.tile([P, H], F32, tag="hps")
                for i, xt in enumerate([xp_t[t][:], x1_t1[t][:, :H], x2_t1[t][:]]):
                    tr = ps.tile([P, P], BF16, tag="trA")
                    nc.tensor.transpose(out=tr[:], in_=xt, identity=ident[:])
                    xT = sb.tile([P, P], BF16, tag="xcatA", name=f"m1T{i}")
                    nc.vector.tensor_copy(xT[:], tr[:])
                    nc.tensor.matmul(out=hps[:], lhsT=xT[:],
                                     rhs=w1_sb[:, i * H:(i + 1) * H],
                                     start=(i == 0), stop=(i == 2))
                hsb = sb.tile([P, H], BF16, tag="hsb", name="m1h")
                nc.vector.tensor_tensor(out=hsb[:], in0=hps[:],
                                        in1=b1_sb[:], op=OP.add)
                nc.vector.tensor_scalar_max(hsb[:], hsb[:], 0.0)
                tr = ps.tile([P, P], BF16, tag="trA")
                nc.tensor.transpose(out=tr[:], in_=hsb[:], identity=ident[:])
                tmax = sb.tile([P, 1], F32, tag="tmax", name="m1t")
                nc.vector.tensor_reduce(out=tmax[:], in_=tr[:], axis=AX.X, op=OP.max)
                nc.vector.tensor_tensor(out=gmax[:], in0=gmax[:], in1=tmax[:], op=OP.max)

            nc.sync.dma_start(gmax_in[:], gmax[:])
            nc.gpsimd.collective_compute(
                "AllReduce", OP.max, replica_groups=groups,
                ins=[gmax_in[:].opt()], outs=[gmax_out[:].opt()])
            gmax2 = sb1.tile([P, 1], F32, name="gmax2")
            nc.sync.dma_start(gmax2[:], gmax_out[:])

            zps = ps.tile([1, OUT], F32, tag="seg")
            nc.tensor.matmul(out=zps[:], lhsT=gmax2[:], rhs=wc_sb[:, :OUT],
                             start=True, stop=True)
            z = sb1.tile([1, OUT], F32, name="zrow")
            nc.vector.tensor_tensor(out=z[:], in0=zps[:], in1=bc_sb[:], op=OP.add)
            m = sb1.tile([1, 1], F32, name="mrow")
            nc.vector.tensor_reduce(out=m[:], in_=z[:], axis=AX.X, op=OP.max)
            zc = sb1.tile([1, OUT], F32, name="zcrow")
            nc.vector.tensor_tensor(out=zc[:], in0=z[:],
                                    in1=m[:].to_broadcast([1, OUT]), op=OP.subtract)
            ez = sb1.tile([1, OUT], F32, name="ezrow")
            nc.scalar.activation(ez[:], zc[:], AF.Exp)
            s = sb1.tile([1, 1], F32, name="srow")
            nc.vector.tensor_reduce(out=s[:], in_=ez[:], axis=AX.X, op=OP.add)
            ls = sb1.tile([1, 1], F32, name="lsrow")
            nc.scalar.activation(ls[:], s[:], AF.Ln)
            yv = sb1.tile([1, OUT], F32, name="yrow")
            nc.vector.tensor_tensor(out=yv[:], in0=zc[:],
                                    in1=ls[:].to_broadcast([1, OUT]), op=OP.subtract)
            nc.sync.dma_start(y_d[:], yv[:])

    nc.compile()
    return nc


# --------------------------------------------------------------------------
# host entry
# --------------------------------------------------------------------------

_CACHE = {}


def prepare(feat, src0, dst0, src1, dst1, W0, b0, W1, b1, Wc, bc, NC=8):
    N0, IN = feat.shape
    N1 = N0 // 2
    H = W0.shape[1]
    OUT = Wc.shape[1]
    SH0, SH1 = N0 // NC, N1 // NC
    T0, T1, TP = SH0 // P, SH1 // P, SH0 // 2 // P

    feat = np.asarray(feat, np.float32)
    src0 = np.asarray(src0)
    dst0 = np.asarray(dst0)
    src1 = np.asarray(src1)
    dst1 = np.asarray(dst1)

    CHt0, pc0 = _prep_layer(src0, dst0, N0, NC, True)
    CHt1, pc1 = _prep_layer(src1, dst1, N1, NC, False)

    deg0 = np.bincount(dst0, minlength=N0).astype(np.float32)
    dinv0 = 1.0 / np.sqrt(np.maximum(deg0, 1.0))
    deg1 = np.bincount(dst1, minlength=N1).astype(np.float32)
    dinv1 = 1.0 / np.sqrt(np.maximum(deg1, 1.0))

    key = (N0, IN, H, OUT, NC, tuple(CHt0), tuple(CHt1))
    if key not in _CACHE:
        cfg = {"N0": N0, "N1": N1, "IN": IN, "H": H, "OUT": OUT, "NC": NC,
               "CHt0": CHt0, "CHt1": CHt1}
        _CACHE[key] = build_program(cfg)
    nc = _CACHE[key]

    iota_np = np.broadcast_to(np.arange(P, dtype=np.float32), (P, P)).astype(BF)
    ident_np = np.eye(P, dtype=np.float32).astype(BF)

    in_maps = []
    for c in range(NC):
        m = {
            "feat_sh": feat[c * SH0:(c + 1) * SH0],
            "feat16_sh": feat[c * SH0:(c + 1) * SH0].astype(BF),
            "dinv0_pm": _dinv_cols(dinv0, c * SH0, SH0),
            "idx0": pc0[c]["idx"],
            "dl0": pc0[c]["dl"],
            "cnt0": pc0[c]["cnt"],
            "idx1": pc1[c]["idx"],
            "dl1": pc1[c]["dl"],
            "cnt1": pc1[c]["cnt"],
            "nnp0": _interleave(-_dinv_cols(dinv0, c * SH0, SH0),
                                -(_dinv_cols(dinv0, c * SH0, SH0) ** 2)),
            "n2dv0": -2.0 * _dinv_cols(dinv0, c * SH0, SH0),
            "pdv1s": _dinv_cols(dinv1, c * SH0 // 2, SH0 // 2),
            "nnp1": _interleave(-_dinv_cols(dinv1, c * SH1, SH1),
                                -(_dinv_cols(dinv1, c * SH1, SH1) ** 2)),
            "n2dv1": -2.0 * _dinv_cols(dinv1, c * SH1, SH1),
            "W0b": np.asarray(W0, np.float32).astype(BF),
            "b0r": np.broadcast_to(np.asarray(b0, np.float32), (P, H)).copy(),
            "W1b": np.asarray(W1, np.float32).astype(BF),
            "b1r": np.broadcast_to(np.asarray(b1, np.float32), (P, H)).copy(),
            "Wc": np.asarray(Wc, np.float32),
            "bcr": np.asarray(bc, np.float32).reshape(1, OUT),
            "iota16": iota_np,
            "ident16": ident_np,
        }
        in_maps.append(m)

    return nc, in_maps


def run(feat, src0, dst0, src1, dst1, W0, b0, W1, b1, Wc, bc, NC=8, **rkw):
    nc, in_maps = prepare(feat, src0, dst0, src1, dst1, W0, b0, W1, b1, Wc, bc, NC)
    res = bass_utils.run_bass_kernel_spmd(nc, in_maps, core_ids=list(range(NC)), **rkw)
    return res.results[0]["y"], res


def kernel(**inputs):
    y, _ = run(**inputs)
    return y
